# revision 8
# baseline (speedup 1.0000x reference)
"""DecoderLSTM (Bahdanau attention + LSTM + vocab fc) on 8 Trainium2 cores.

Sharding: data-parallel over batch (64 -> 8 rows/core); the sequential scan
stays local per core; zero collectives. Host only shards/casts/transposes
inputs and reassembles the output.

Per-core (b=8 rows, N=196, H=E=A=512, V=20000, T=19 steps):
  setup:  X_proj[t*8+b,:] = [emb(x);1] @ [W_ih_x;b_ih+b_hh]^T -> DRAM scratch
          enc_proj^T[a,(b,n)] = enc_W @ enc^T (+enc_b)        -> SBUF resident
  step t: dec = 2h @ (.5 dec_W)^T; dec^T via PE transpose (+dec_b)
          X = tanh(enc_proj^T + bcast dec^T)   [128,1568] x4  (DVE/GPSIMD+ACT)
          scores: M=8-redundant matmul w_e . X; softmax via exp (fused row
          sums) on the valid diagonal rows; attw^T via PE transpose
          ctx_b = attw_b . enc_b (M=8 redundant, row b valid) -> ctx^T
          gates = [ctx;2h] @ Wc^T + X_proj[t]  (identity-matmul accumulate)
          pointwise with sigmoid(x)=(tanh(x/2)+1)/2; states C2=2c, H2=2h
          (factor 2 folded into host-prescaled 0.5*{W_hh, dec_W, fc_W})
  fc:     logits = (H2_all)^T @ (.5 fc_W)^T + fc_b, 500-col vocab chunks
"""

import numpy as np

import concourse.bass as bass
import concourse.bacc as bacc
import concourse.tile as tile
from concourse import mybir
from concourse.bass_utils import run_bass_kernel_spmd

F16 = mybir.dt.float16
F32 = mybir.dt.float32

B, N, H, E, A, V, L = 64, 196, 512, 512, 512, 20000, 20
T = L - 1            # 19 decode steps
NC = 8               # cores
BS = B // NC         # 8 batch rows per core
BN = BS * N          # 1568
BT = T * BS          # 152 rows, t-major (row = t*8 + b)
VC = 500             # fc vocab chunk width
NCH = V // VC        # 40

# gate reorder [i,f,g,o] -> [i,f,o,g] so tanh(0.5*x) covers cols 0:1536
PERM = np.concatenate([np.arange(0, H), np.arange(H, 2 * H),
                       np.arange(3 * H, 4 * H), np.arange(2 * H, 3 * H)])

TANH = mybir.ActivationFunctionType.Tanh
EXP = mybir.ActivationFunctionType.Exp
ADD = mybir.AluOpType.add
MULT = mybir.AluOpType.mult


def prep_core(core, inputs):
    """Per-core numpy input dict (shard + transpose + cast only)."""
    f32 = np.float32
    bsl = slice(core * BS, (core + 1) * BS)
    enc = np.asarray(inputs["encoder_outputs"][bsl], f32)      # [8,196,512]

    enc_t = np.ascontiguousarray(enc.reshape(BN, H).T).astype(np.float16)
    enc_r = np.zeros((2 * BS, 128, H), np.float16)
    for b in range(BS):
        enc_r[2 * b, :128] = enc[b, :128]
        enc_r[2 * b + 1, :N - 128] = enc[b, 128:]

    caps = np.asarray(inputs["captions"][bsl])[:, :T]          # [8,19]
    es = np.asarray(inputs["emb"], f32)[caps]                  # [8,19,512]
    emb_flat = es.transpose(1, 0, 2).reshape(BT, E)            # t-major rows
    emb_cat = np.concatenate(
        [emb_flat.T, np.ones((1, BT), f32)], 0).astype(np.float16)

    wih = np.asarray(inputs["W_ih"], f32)[PERM]                # [2048,1024]
    whh = np.asarray(inputs["W_hh"], f32)[PERM]
    bias = (np.asarray(inputs["b_ih"], f32) +
            np.asarray(inputs["b_hh"], f32))[PERM]
    wihxb_t = np.concatenate(
        [wih[:, :E].T, bias[None, :]], 0).astype(np.float16)   # [513,2048]
    wc_t = np.concatenate(
        [wih[:, E:].T, 0.5 * whh.T], 0).astype(np.float16)     # [1024,2048]

    dec_wt = (0.5 * np.asarray(inputs["dec_W"], f32).T).astype(np.float16)
    enc_wt = np.ascontiguousarray(
        np.asarray(inputs["enc_W"], f32).T).astype(np.float16)  # [H,A]
    decb = np.ascontiguousarray(
        np.asarray(inputs["dec_b"], f32).reshape(4, 128).T)     # [128,4]
    encb = np.ascontiguousarray(
        np.asarray(inputs["enc_b"], f32).reshape(4, 128).T)
    ew = np.ascontiguousarray(
        np.asarray(inputs["energy_W"], f32)[0].reshape(4, 128).T
    ).astype(np.float16)                                        # [128,4]
    fcw_t = np.ascontiguousarray(
        0.5 * np.asarray(inputs["fc_W"], f32).T).astype(np.float16)
    fcb = np.ascontiguousarray(
        np.asarray(inputs["fc_b"], f32)[None, :]).astype(np.float16)
    id8 = np.eye(8, dtype=np.float16)

    return {"enc_t": enc_t, "enc_r": enc_r, "emb_cat": emb_cat,
            "wihxb_t": wihxb_t, "wc_t": wc_t, "dec_wt": dec_wt,
            "enc_wt": enc_wt, "decb": decb, "encb": encb, "ew": ew,
            "fcw_t": fcw_t, "fcb": fcb, "id8": id8}


def _bcast(ap, n):
    """Append an innermost step-0 (broadcast) dim of size n to an AP."""
    return bass.AP(tensor=ap.tensor, offset=ap.offset,
                   ap=list(ap.ap) + [[0, n]])


def build_program():
    nc = bacc.Bacc("TRN2", target_bir_lowering=False, debug=False,
                   num_devices=NC)
    d_enc_t = nc.dram_tensor("enc_t", [H, BN], F16, kind="ExternalInput")
    d_enc_r = nc.dram_tensor("enc_r", [2 * BS, 128, H], F16,
                             kind="ExternalInput")
    d_emb = nc.dram_tensor("emb_cat", [E + 1, BT], F16, kind="ExternalInput")
    d_wx = nc.dram_tensor("wihxb_t", [E + 1, 4 * H], F16,
                          kind="ExternalInput")
    d_wc = nc.dram_tensor("wc_t", [2 * H, 4 * H], F16, kind="ExternalInput")
    d_dwt = nc.dram_tensor("dec_wt", [H, A], F16, kind="ExternalInput")
    d_ewt = nc.dram_tensor("enc_wt", [H, A], F16, kind="ExternalInput")
    d_decb = nc.dram_tensor("decb", [128, 4], F32, kind="ExternalInput")
    d_encb = nc.dram_tensor("encb", [128, 4], F32, kind="ExternalInput")
    d_ew = nc.dram_tensor("ew", [128, 4], F16, kind="ExternalInput")
    d_fcw = nc.dram_tensor("fcw_t", [H, V], F16, kind="ExternalInput")
    d_fcb = nc.dram_tensor("fcb", [1, V], F16, kind="ExternalInput")
    d_id8 = nc.dram_tensor("id8", [8, 8], F16, kind="ExternalInput")
    d_out = nc.dram_tensor("logits", [BT, V], F32, kind="ExternalOutput")
    d_xp = nc.dram_tensor("xproj", [BT, 4 * H], F16, kind="Internal")

    with tile.TileContext(nc) as tc:
        _build_body(nc, tc, d_enc_t, d_enc_r, d_emb, d_wx, d_wc, d_dwt,
                    d_ewt, d_decb, d_encb, d_ew, d_fcw, d_fcb, d_id8,
                    d_out, d_xp)
    nc.compile()
    return nc


def _build_body(nc, tc, d_enc_t, d_enc_r, d_emb, d_wx, d_wc, d_dwt, d_ewt,
                d_decb, d_encb, d_ew, d_fcw, d_fcb, d_id8, d_out, d_xp):
    with tc.tile_pool(name="res", bufs=1) as res:
        # -------- residents --------
        ept = res.tile([128, 4, BN], F16)        # enc_proj^T a-tiles
        wc = res.tile([128, 8, 4 * H], F16)
        enr = res.tile([128, 2 * BS, H], F16)
        dwt = res.tile([128, 4, A], F16)
        decb = res.tile([128, 4], F32)
        encb = res.tile([128, 4], F32)
        ewm = res.tile([128, 4, 4, 4], F16)    # diag: [:,at,bl,bl]=ew
        atm = res.tile([128, 2, 4, 4], F16)    # per-half diag attw (n<128)
        at2m = res.tile([128, 2, 4, 4], F16)   # per-half diag (n=128:196)
        id8 = res.tile([8, 8], F16)
        hallt = res.tile([128, 4, BT], F16)      # H2^T, all steps
        h0 = res.tile([128, 4, 8], F16)
        c2 = res.tile([8, H], F32)
        ones = res.tile([1, 128], F16)
        NPRE = 16                                 # prefetched fc chunks
        fcpre = res.tile([128, NPRE, 4, VC], F16)

        nc.sync.dma_start(out=decb[:, :], in_=d_decb[:, :])
        nc.sync.dma_start(out=encb[:, :], in_=d_encb[:, :])
        nc.sync.dma_start(out=id8[:, :], in_=d_id8[:, :])
        ew_col = res.tile([128, 4], F16)
        nc.sync.dma_start(out=ew_col[:, :], in_=d_ew[:, :])
        nc.vector.memset(ewm[:, :, :, :], 0.0)
        nc.vector.memset(atm[:, :, :, :], 0.0)
        nc.vector.memset(at2m[:, :, :, :], 0.0)
        for at in range(4):
            col = ew_col[:, at:at + 1]
            dg = ewm[:, at, :, :]
            nc.vector.tensor_copy(
                out=bass.AP(tensor=dg.tensor, offset=dg.offset,
                            ap=[dg.ap[0], [5, 4]]),
                in_=bass.AP(tensor=col.tensor, offset=col.offset,
                            ap=[col.ap[0], [0, 4]]))
        nc.vector.memset(h0[:, :, :], 0.0)
        nc.vector.memset(c2[:, :], 0.0)
        nc.vector.memset(ones[:, :], 1.0)

        # -------- setup: X_proj to DRAM scratch --------
        with tc.tile_pool(name="sx", bufs=1) as sx, \
             tc.tile_pool(name="sxp", bufs=2, space="PSUM") as sxp, \
             tc.tile_pool(name="sxs", bufs=3) as sxs:
            ec = sx.tile([128, 5, BT], F16)
            wx = sx.tile([128, 5, 4 * H], F16)
            for k in range(4):
                nc.sync.dma_start(out=ec[:, k, :],
                                  in_=d_emb[k * 128:(k + 1) * 128, :])
                nc.sync.dma_start(out=wx[:, k, :],
                                  in_=d_wx[k * 128:(k + 1) * 128, :])
            nc.sync.dma_start(out=ec[0:1, 4, :], in_=d_emb[512:513, :])
            nc.sync.dma_start(out=wx[0:1, 4, :], in_=d_wx[512:513, :])
            for m in range(2):
                mr = 128 if m == 0 else BT - 128
                for ch in range(4):
                    pt = sxp.tile([128, 512], F32, tag="sxp")
                    for k in range(5):
                        kr = 128 if k < 4 else 1
                        nc.tensor.matmul(
                            pt[0:mr, :],
                            ec[0:kr, k, m * 128:m * 128 + mr],
                            wx[0:kr, k, ch * 512:(ch + 1) * 512],
                            start=(k == 0), stop=(k == 4))
                    st = sxs.tile([128, 512], F16, tag="st")
                    nc.vector.tensor_copy(out=st[0:mr, :], in_=pt[0:mr, :])
                    nc.sync.dma_start(
                        out=d_xp[m * 128:m * 128 + mr,
                                 ch * 512:(ch + 1) * 512],
                        in_=st[0:mr, :])

        # -------- setup: enc_proj^T (+enc_b) --------
        with tc.tile_pool(name="se", bufs=1) as se, \
             tc.tile_pool(name="sep", bufs=2, space="PSUM") as sep:
            et = se.tile([128, 4, BN], F16)
            ewt = se.tile([128, 4, A], F16)
            for k in range(4):
                nc.sync.dma_start(out=et[:, k, :],
                                  in_=d_enc_t[k * 128:(k + 1) * 128, :])
                nc.sync.dma_start(out=ewt[:, k, :],
                                  in_=d_ewt[k * 128:(k + 1) * 128, :])
            for at in range(4):                      # a-tile = out m-tile
                for ch in range(4):                  # 1568 = 4*392
                    pt = sep.tile([128, 392], F32, tag="sep")
                    for k in range(4):
                        nc.tensor.matmul(
                            pt[:, :],
                            ewt[:, k, at * 128:(at + 1) * 128],
                            et[:, k, ch * 392:(ch + 1) * 392],
                            start=(k == 0), stop=(k == 3))
                    nc.vector.tensor_scalar_add(
                        out=ept[:, at, ch * 392:(ch + 1) * 392],
                        in0=pt[:, :], scalar1=encb[:, at:at + 1])

        # load remaining residents
        for k in range(8):
            nc.sync.dma_start(out=wc[:, k, :],
                              in_=d_wc[k * 128:(k + 1) * 128, :])
        for j in range(2 * BS):
            nc.sync.dma_start(out=enr[:, j, :], in_=d_enc_r[j, :, :])
        for k in range(4):
            nc.sync.dma_start(out=dwt[:, k, :],
                              in_=d_dwt[k * 128:(k + 1) * 128, :])
        for ch in range(NPRE):                    # stream during recurrence
            for k in range(4):
                nc.sync.dma_start(
                    out=fcpre[:, ch, k, :],
                    in_=d_fcw[k * 128:(k + 1) * 128,
                              ch * VC:(ch + 1) * VC])

        # -------- recurrence --------
        with tc.tile_pool(name="psm", bufs=4, space="PSUM") as psm, \
             tc.tile_pool(name="psg", bufs=1, space="PSUM") as psgp, \
             tc.tile_pool(name="stp", bufs=2) as stp, \
             tc.tile_pool(name="xp", bufs=2) as xp, \
             tc.tile_pool(name="gxp", bufs=2) as gxp:
            for t in range(T):
                hprev = (lambda at: h0[:, at, :]) if t == 0 else \
                    (lambda at, _t=t: hallt[:, at, (_t - 1) * 8:(_t - 1) * 8 + 8])

                # ---- dec = 2h @ (.5 dec_W)^T ----
                pd = psm.tile([8, 512], F32, tag="sm")
                for k in range(4):
                    nc.tensor.matmul(pd[:, :], hprev(k), dwt[:, k, :],
                                     start=(k == 0), stop=(k == 3))
                dec = stp.tile([8, 512], F16, tag="dec")
                nc.vector.tensor_copy(out=dec[:, :], in_=pd[:, :])

                # ---- dec^T (+dec_b) ----
                dect = stp.tile([128, 4, 8], F16, tag="dect")
                for at in range(4):
                    ptr = psm.tile([128, 8], F16, tag="sm")
                    nc.tensor.transpose(ptr[:, :],
                                        dec[:, at * 128:(at + 1) * 128],
                                        id8[:, :])
                    nc.vector.tensor_scalar_add(
                        out=dect[:, at, :], in0=ptr[:, :],
                        scalar1=decb[:, at:at + 1])

                # ---- gates: h-part first (only needs h(t-1)) ----
                gx = gxp.tile([8, 4 * H], F16, tag="gx")
                nc.sync.dma_start(out=gx[:, :],
                                  in_=d_xp[t * 8:(t + 1) * 8, :])
                psg = psgp.tile([8, 4 * H], F32, tag="gates")
                for ch in range(4):
                    sl = slice(ch * 512, (ch + 1) * 512)
                    for k in range(4):
                        nc.tensor.matmul(psg[:, sl], hprev(k),
                                         wc[:, 4 + k, sl],
                                         start=(k == 0), stop=False)
                    nc.tensor.matmul(psg[:, sl], id8[:, :], gx[:, sl],
                                     start=False, stop=False)

                # ---- attention, pipelined over two half-batches ----
                xts = []
                for at in range(4):
                    xts.append(xp.tile([128, BN], F16, tag=f"x{at}",
                                       name=f"xt{t}_{at}"))
                ct = stp.tile([128, 4, 8], F16, tag="ct")
                for h in range(2):
                    hsl = slice(h * 4 * N, (h + 1) * 4 * N)
                    for at in range(4):
                        xt = xts[at]
                        eng = nc.vector if (at + 2 * h) % 4 < 2 else nc.gpsimd
                        eng.tensor_add(
                            out=xt[:, hsl].rearrange(
                                "p (b n) -> p b n", n=N),
                            in0=ept[:, at, hsl].rearrange(
                                "p (b n) -> p b n", n=N),
                            in1=_bcast(dect[:, at, h * 4:(h + 1) * 4], N))
                        nc.scalar.activation(out=xt[:, hsl],
                                             in_=xt[:, hsl], func=TANH)
                    psc = psm.tile([4, N], F32, tag="sm")
                    for bl in range(4):
                        b = 4 * h + bl
                        for at in range(4):
                            nc.tensor.matmul(
                                psc[:, :], ewm[:, at, bl, :],
                                xts[at][:, b * N:(b + 1) * N],
                                start=(bl == 0 and at == 0),
                                stop=(bl == 3 and at == 3))
                    exps = stp.tile([4, N], F32, tag=f"exps{h}")
                    zs = stp.tile([4, 1], F32, tag=f"zs{h}")
                    nc.scalar.activation(out=exps[:, :], in_=psc[:, :],
                                         func=EXP, accum_out=zs[:, 0:1])
                    rz = stp.tile([4, 1], F32, tag=f"rz{h}")
                    nc.vector.reciprocal(out=rz[:, :], in_=zs[:, :])
                    atw = stp.tile([4, N], F16, tag=f"atw{h}")
                    nc.vector.tensor_scalar_mul(
                        out=atw[:, :], in0=exps[:, :], scalar1=rz[:, :])

                    p1 = psm.tile([128, 4], F16, tag="sm")
                    nc.tensor.transpose(p1[:, :], atw[:, 0:128],
                                        id8[0:4, 0:4])
                    dg = atm[:, h, :, :]
                    nc.vector.tensor_copy(
                        out=bass.AP(tensor=dg.tensor, offset=dg.offset,
                                    ap=[dg.ap[0], [5, 4]]),
                        in_=p1[:, :])
                    p2 = psm.tile([128, 4], F16, tag="sm")
                    nc.tensor.transpose(p2[0:N - 128, :], atw[:, 128:N],
                                        id8[0:4, 0:4])
                    d2 = at2m[0:N - 128, h, :, :]
                    nc.vector.tensor_copy(
                        out=bass.AP(tensor=d2.tensor, offset=d2.offset,
                                    ap=[d2.ap[0], [5, 4]]),
                        in_=p2[0:N - 128, :])

                    pc = psm.tile([4, 512], F32, tag="sm")
                    for bl in range(4):
                        b = 4 * h + bl
                        nc.tensor.matmul(pc[:, :], atm[:, h, bl, :],
                                         enr[:, 2 * b, :],
                                         start=(bl == 0), stop=False)
                        nc.tensor.matmul(pc[:, :],
                                         at2m[0:N - 128, h, bl, :],
                                         enr[0:N - 128, 2 * b + 1, :],
                                         start=False, stop=(bl == 3))
                    ctxr = stp.tile([4, H], F16, tag=f"ctxr{h}")
                    nc.vector.tensor_copy(out=ctxr[:, :], in_=pc[:, :])
                    for at in range(4):
                        ptr = psm.tile([128, 4], F16, tag="sm")
                        nc.tensor.transpose(
                            ptr[:, :], ctxr[:, at * 128:(at + 1) * 128],
                            id8[0:4, 0:4])
                        nc.vector.tensor_copy(
                            out=ct[:, at, h * 4:(h + 1) * 4],
                            in_=ptr[:, :])

                # ---- gates: ctx-part ----
                for ch in range(4):
                    sl = slice(ch * 512, (ch + 1) * 512)
                    for k in range(4):
                        nc.tensor.matmul(psg[:, sl], ct[:, k, :],
                                         wc[:, k, sl],
                                         start=False, stop=(k == 3))

                # ---- pointwise (i,f,o,g; sigmoid via tanh) ----
                th = stp.tile([8, 3 * H], F16, tag="th")
                nc.scalar.activation(out=th[:, :], in_=psg[:, 0:3 * H],
                                     func=TANH, scale=0.5)
                thg = stp.tile([8, H], F16, tag="thg")
                nc.scalar.activation(out=thg[:, :], in_=psg[:, 3 * H:4 * H],
                                     func=TANH)
                a2 = stp.tile([8, H], F32, tag="a2")
                nc.vector.scalar_tensor_tensor(
                    out=a2[:, :], in0=th[:, H:2 * H], scalar=1.0,
                    in1=c2[:, :], op0=ADD, op1=MULT)
                bb = stp.tile([8, H], F32, tag="bb")
                nc.vector.scalar_tensor_tensor(
                    out=bb[:, :], in0=th[:, 0:H], scalar=1.0,
                    in1=thg[:, :], op0=ADD, op1=MULT)
                nc.vector.scalar_tensor_tensor(
                    out=c2[:, :], in0=a2[:, :], scalar=0.5,
                    in1=bb[:, :], op0=MULT, op1=ADD)
                thc = stp.tile([8, H], F32, tag="thc")
                nc.scalar.activation(out=thc[:, :], in_=c2[:, :],
                                     func=TANH, scale=0.5)
                h2r = stp.tile([8, H], F16, tag="h2r")
                nc.vector.scalar_tensor_tensor(
                    out=h2r[:, :], in0=th[:, 2 * H:3 * H], scalar=1.0,
                    in1=thc[:, :], op0=ADD, op1=MULT)

                # ---- h^T into hallt[:, :, t*8:(t+1)*8] ----
                for at in range(4):
                    ptr = psm.tile([128, 8], F16, tag="sm")
                    nc.tensor.transpose(ptr[:, :],
                                        h2r[:, at * 128:(at + 1) * 128],
                                        id8[:, :])
                    nc.vector.tensor_copy(
                        out=hallt[:, at, t * 8:(t + 1) * 8], in_=ptr[:, :])

        # -------- fc --------
        with tc.tile_pool(name="fw", bufs=8) as fwp, \
             tc.tile_pool(name="fb", bufs=3) as fbp, \
             tc.tile_pool(name="fo", bufs=3) as fop, \
             tc.tile_pool(name="pf", bufs=3, space="PSUM") as pfp:
            for ch in range(NCH):
                vsl = slice(ch * VC, (ch + 1) * VC)
                fbc = fbp.tile([1, VC], F16, tag="fb")
                nc.sync.dma_start(out=fbc[:, :], in_=d_fcb[0:1, vsl])
                if ch < NPRE:
                    fws = [fcpre[:, ch, k, :] for k in range(4)]
                else:
                    fws = []
                    for k in range(4):
                        fw = fwp.tile([128, VC], F16, tag="fw")
                        nc.sync.dma_start(
                            out=fw[:, :],
                            in_=d_fcw[k * 128:(k + 1) * 128, vsl])
                        fws.append(fw)
                for m in range(2):
                    mr = 128 if m == 0 else BT - 128
                    pf = pfp.tile([128, VC], F32, tag="pf")
                    for k in range(4):
                        nc.tensor.matmul(
                            pf[0:mr, :],
                            hallt[:, k, m * 128:m * 128 + mr],
                            fws[k][:, :], start=(k == 0), stop=False)
                    nc.tensor.matmul(pf[0:mr, :], ones[0:1, 0:mr],
                                     fbc[0:1, :], start=False, stop=True)
                    fo = fop.tile([128, VC], F32, tag="fo")
                    nc.vector.tensor_copy(out=fo[0:mr, :], in_=pf[0:mr, :])
                    nc.sync.dma_start(
                        out=d_out[m * 128:m * 128 + mr, vsl],
                        in_=fo[0:mr, :])


_PROGRAM = None


def kernel(**inputs) -> np.ndarray:
    global _PROGRAM
    if _PROGRAM is None:
        _PROGRAM = build_program()
    in_maps = [prep_core(c, inputs) for c in range(NC)]
    res = run_bass_kernel_spmd(_PROGRAM, in_maps, core_ids=list(range(NC)))
    out = np.zeros((B, L, V), np.float32)
    for c in range(NC):
        lg = res.results[c]["logits"].reshape(T, BS, V)
        out[c * BS:(c + 1) * BS, 1:, :] = lg.transpose(1, 0, 2)
    return out


if __name__ == "__main__":
    import reference
    ins = {k: np.asarray(v) for k, v in reference.setup_inputs().items()}
    got = kernel(**ins)
    exp = np.asarray(reference.reference(**reference.setup_inputs()))
    err = np.abs(got - exp).max() / (np.abs(exp).max() + 1e-12)
    print("Relative error:", err)


# revision 15
# speedup vs baseline: 1.1163x; 1.1163x over previous
"""DecoderLSTM (Bahdanau attention + LSTM + vocab fc) on 8 Trainium2 cores.

Sharding: data-parallel over batch (64 -> 8 rows/core); the sequential scan
stays local per core; zero collectives. Host only shards/casts/transposes
inputs and reassembles the output.

Per-core (b=8 rows, N=196, H=E=A=512, V=20000, T=19 steps):
  setup:  X_proj[t*8+b,:] = [emb(x);1] @ [W_ih_x;b_ih+b_hh]^T -> DRAM scratch
          enc_proj^T[a,(b,n)] = enc_W @ enc^T (+enc_b)        -> SBUF resident
  step t: dec = 2h @ (.5 dec_W)^T; dec^T via PE transpose (+dec_b)
          X = tanh(enc_proj^T + bcast dec^T)   [128,1568] x4  (DVE/GPSIMD+ACT)
          scores: M=8-redundant matmul w_e . X; softmax via exp (fused row
          sums) on the valid diagonal rows; attw^T via PE transpose
          ctx_b = attw_b . enc_b (M=8 redundant, row b valid) -> ctx^T
          gates = [ctx;2h] @ Wc^T + X_proj[t]  (identity-matmul accumulate)
          pointwise with sigmoid(x)=(tanh(x/2)+1)/2; states C2=2c, H2=2h
          (factor 2 folded into host-prescaled 0.5*{W_hh, dec_W, fc_W})
  fc:     logits = (H2_all)^T @ (.5 fc_W)^T + fc_b, 500-col vocab chunks
"""

import numpy as np

import concourse.bass as bass
import concourse.bacc as bacc
import concourse.tile as tile
from concourse import mybir
from concourse.bass_utils import run_bass_kernel_spmd

F16 = mybir.dt.float16
F32 = mybir.dt.float32

B, N, H, E, A, V, L = 64, 196, 512, 512, 512, 20000, 20
T = L - 1            # 19 decode steps
NC = 8               # cores
BS = B // NC         # 8 batch rows per core
BN = BS * N          # 1568
BT = T * BS          # 152 rows, t-major (row = t*8 + b)
VC = 500             # fc vocab chunk width
NCH = V // VC        # 40

# gate reorder [i,f,g,o] -> [i,f,o,g] so tanh(0.5*x) covers cols 0:1536
PERM = np.concatenate([np.arange(0, H), np.arange(H, 2 * H),
                       np.arange(3 * H, 4 * H), np.arange(2 * H, 3 * H)])

TANH = mybir.ActivationFunctionType.Tanh
EXP = mybir.ActivationFunctionType.Exp
ADD = mybir.AluOpType.add
MULT = mybir.AluOpType.mult


def prep_core(core, inputs):
    """Per-core numpy input dict (shard + transpose + cast only)."""
    f32 = np.float32
    bsl = slice(core * BS, (core + 1) * BS)
    enc = np.asarray(inputs["encoder_outputs"][bsl], f32)      # [8,196,512]

    enc_t = np.ascontiguousarray(enc.reshape(BN, H).T).astype(np.float16)
    enc_r = np.zeros((2 * BS, 128, H), np.float16)
    for b in range(BS):
        enc_r[2 * b, :128] = enc[b, :128]
        enc_r[2 * b + 1, :N - 128] = enc[b, 128:]

    caps = np.asarray(inputs["captions"][bsl])[:, :T]          # [8,19]
    es = np.asarray(inputs["emb"], f32)[caps]                  # [8,19,512]
    emb_flat = es.transpose(1, 0, 2).reshape(BT, E)            # t-major rows
    emb_cat = np.concatenate(
        [emb_flat.T, np.ones((1, BT), f32)], 0).astype(np.float16)

    wih = np.asarray(inputs["W_ih"], f32)[PERM]                # [2048,1024]
    whh = np.asarray(inputs["W_hh"], f32)[PERM]
    bias = (np.asarray(inputs["b_ih"], f32) +
            np.asarray(inputs["b_hh"], f32))[PERM]
    wihxb_t = np.concatenate(
        [wih[:, :E].T, bias[None, :]], 0).astype(np.float16)   # [513,2048]
    wc_t = np.concatenate(
        [wih[:, E:].T, 0.5 * whh.T], 0).astype(np.float16)     # [1024,2048]

    dec_wt = (0.5 * np.asarray(inputs["dec_W"], f32).T).astype(np.float16)
    enc_wt = np.ascontiguousarray(
        np.asarray(inputs["enc_W"], f32).T).astype(np.float16)  # [H,A]
    decb = np.ascontiguousarray(
        np.asarray(inputs["dec_b"], f32).reshape(4, 128).T)     # [128,4]
    encb = np.ascontiguousarray(
        np.asarray(inputs["enc_b"], f32).reshape(4, 128).T)
    ew = np.ascontiguousarray(
        np.asarray(inputs["energy_W"], f32)[0].reshape(4, 128).T
    ).astype(np.float16)                                        # [128,4]
    fcw_t = np.ascontiguousarray(
        0.5 * np.asarray(inputs["fc_W"], f32).T).astype(np.float16)
    fcb = np.ascontiguousarray(
        np.asarray(inputs["fc_b"], f32)[None, :]).astype(np.float16)
    id8 = np.eye(8, dtype=np.float16)

    return {"enc_t": enc_t, "enc_r": enc_r, "emb_cat": emb_cat,
            "wihxb_t": wihxb_t, "wc_t": wc_t, "dec_wt": dec_wt,
            "enc_wt": enc_wt, "decb": decb, "encb": encb, "ew": ew,
            "fcw_t": fcw_t, "fcb": fcb, "id8": id8}


def _bcast(ap, n):
    """Append an innermost step-0 (broadcast) dim of size n to an AP."""
    return bass.AP(tensor=ap.tensor, offset=ap.offset,
                   ap=list(ap.ap) + [[0, n]])


def build_program():
    nc = bacc.Bacc("TRN2", target_bir_lowering=False, debug=False,
                   num_devices=NC)
    d_enc_t = nc.dram_tensor("enc_t", [H, BN], F16, kind="ExternalInput")
    d_enc_r = nc.dram_tensor("enc_r", [2 * BS, 128, H], F16,
                             kind="ExternalInput")
    d_emb = nc.dram_tensor("emb_cat", [E + 1, BT], F16, kind="ExternalInput")
    d_wx = nc.dram_tensor("wihxb_t", [E + 1, 4 * H], F16,
                          kind="ExternalInput")
    d_wc = nc.dram_tensor("wc_t", [2 * H, 4 * H], F16, kind="ExternalInput")
    d_dwt = nc.dram_tensor("dec_wt", [H, A], F16, kind="ExternalInput")
    d_ewt = nc.dram_tensor("enc_wt", [H, A], F16, kind="ExternalInput")
    d_decb = nc.dram_tensor("decb", [128, 4], F32, kind="ExternalInput")
    d_encb = nc.dram_tensor("encb", [128, 4], F32, kind="ExternalInput")
    d_ew = nc.dram_tensor("ew", [128, 4], F16, kind="ExternalInput")
    d_fcw = nc.dram_tensor("fcw_t", [H, V], F16, kind="ExternalInput")
    d_fcb = nc.dram_tensor("fcb", [1, V], F16, kind="ExternalInput")
    d_id8 = nc.dram_tensor("id8", [8, 8], F16, kind="ExternalInput")
    d_out = nc.dram_tensor("logits", [BT, V], F32, kind="ExternalOutput")
    d_xp = nc.dram_tensor("xproj", [BT, 4 * H], F16, kind="Internal")

    with tile.TileContext(nc) as tc:
        _build_body(nc, tc, d_enc_t, d_enc_r, d_emb, d_wx, d_wc, d_dwt,
                    d_ewt, d_decb, d_encb, d_ew, d_fcw, d_fcb, d_id8,
                    d_out, d_xp)
    nc.compile()
    return nc


def _build_body(nc, tc, d_enc_t, d_enc_r, d_emb, d_wx, d_wc, d_dwt, d_ewt,
                d_decb, d_encb, d_ew, d_fcw, d_fcb, d_id8, d_out, d_xp):
    with tc.tile_pool(name="res", bufs=1) as res:
        # -------- residents --------
        ept = res.tile([128, 4, BN], F16)        # enc_proj^T a-tiles
        wc = res.tile([128, 8, 4 * H], F16)
        enr = res.tile([128, 2 * BS, H], F16)
        dwt = res.tile([128, 4, A], F16)
        decb = res.tile([128, 4], F32)
        encb = res.tile([128, 4], F32)
        ewm = res.tile([128, 4, 4, 4], F16)    # diag: [:,at,bl,bl]=ew
        atm = res.tile([128, 2, 4, 4], F16)    # per-half diag attw (n<128)
        at2m = res.tile([128, 2, 4, 4], F16)   # per-half diag (n=128:196)
        id8 = res.tile([8, 8], F16)
        hallt = res.tile([128, 4, BT], F16)      # H2^T, all steps
        h0 = res.tile([128, 4, 8], F16)
        c2 = res.tile([8, H], F32)
        ones = res.tile([1, 128], F16)
        NPRE = 19                                 # prefetched fc chunks
        fcpre = res.tile([128, NPRE, 4, VC], F16)

        nc.sync.dma_start(out=decb[:, :], in_=d_decb[:, :])
        nc.sync.dma_start(out=encb[:, :], in_=d_encb[:, :])
        nc.sync.dma_start(out=id8[:, :], in_=d_id8[:, :])
        ew_col = res.tile([128, 4], F16)
        nc.sync.dma_start(out=ew_col[:, :], in_=d_ew[:, :])
        nc.vector.memset(ewm[:, :, :, :], 0.0)
        nc.vector.memset(atm[:, :, :, :], 0.0)
        nc.vector.memset(at2m[:, :, :, :], 0.0)
        for at in range(4):
            col = ew_col[:, at:at + 1]
            dg = ewm[:, at, :, :]
            nc.vector.tensor_copy(
                out=bass.AP(tensor=dg.tensor, offset=dg.offset,
                            ap=[dg.ap[0], [5, 4]]),
                in_=bass.AP(tensor=col.tensor, offset=col.offset,
                            ap=[col.ap[0], [0, 4]]))
        nc.vector.memset(h0[:, :, :], 0.0)
        nc.vector.memset(c2[:, :], 0.0)
        nc.vector.memset(ones[:, :], 1.0)

        # -------- setup: X_proj to DRAM scratch --------
        with tc.tile_pool(name="sx", bufs=1) as sx, \
             tc.tile_pool(name="sxp", bufs=2, space="PSUM") as sxp, \
             tc.tile_pool(name="sxs", bufs=3) as sxs:
            ec = sx.tile([128, 5, BT], F16)
            wx = sx.tile([128, 5, 4 * H], F16)
            nc.sync.dma_start(
                out=ec[:, 0:4, :],
                in_=d_emb[0:512, :].rearrange("(k p) t -> p k t", p=128))
            nc.sync.dma_start(
                out=wx[:, 0:4, :],
                in_=d_wx[0:512, :].rearrange("(k p) g -> p k g", p=128))
            nc.sync.dma_start(out=ec[0:1, 4, :], in_=d_emb[512:513, :])
            nc.sync.dma_start(out=wx[0:1, 4, :], in_=d_wx[512:513, :])
            for m in range(2):
                mr = 128 if m == 0 else BT - 128
                for ch in range(4):
                    pt = sxp.tile([128, 512], F32, tag="sxp")
                    for k in range(5):
                        kr = 128 if k < 4 else 1
                        nc.tensor.matmul(
                            pt[0:mr, :],
                            ec[0:kr, k, m * 128:m * 128 + mr],
                            wx[0:kr, k, ch * 512:(ch + 1) * 512],
                            start=(k == 0), stop=(k == 4))
                    st = sxs.tile([128, 512], F16, tag="st")
                    nc.vector.tensor_copy(out=st[0:mr, :], in_=pt[0:mr, :])
                    nc.sync.dma_start(
                        out=d_xp[m * 128:m * 128 + mr,
                                 ch * 512:(ch + 1) * 512],
                        in_=st[0:mr, :])

        # -------- setup: enc_proj^T (+enc_b) --------
        with tc.tile_pool(name="se", bufs=1) as se, \
             tc.tile_pool(name="sep", bufs=2, space="PSUM") as sep:
            et = se.tile([128, 4, BN], F16)
            ewt = se.tile([128, 4, A], F16)
            nc.sync.dma_start(
                out=et[:, :, :],
                in_=d_enc_t[:, :].rearrange("(k p) n -> p k n", p=128))
            nc.sync.dma_start(
                out=ewt[:, :, :],
                in_=d_ewt[:, :].rearrange("(k p) a -> p k a", p=128))
            for at in range(4):                      # a-tile = out m-tile
                for ch in range(4):                  # 1568 = 4*392
                    pt = sep.tile([128, 392], F32, tag="sep")
                    for k in range(4):
                        nc.tensor.matmul(
                            pt[:, :],
                            ewt[:, k, at * 128:(at + 1) * 128],
                            et[:, k, ch * 392:(ch + 1) * 392],
                            start=(k == 0), stop=(k == 3))
                    nc.vector.tensor_scalar_add(
                        out=ept[:, at, ch * 392:(ch + 1) * 392],
                        in0=pt[:, :], scalar1=encb[:, at:at + 1])

        # load remaining residents (batched DMAs)
        nc.sync.dma_start(out=wc[:, :, :],
                          in_=d_wc[:, :].rearrange("(k p) g -> p k g", p=128))
        nc.sync.dma_start(out=enr[:, :, :],
                          in_=d_enc_r[:, :, :].rearrange("j p h -> p j h"))
        nc.sync.dma_start(out=dwt[:, :, :],
                          in_=d_dwt[:, :].rearrange("(k p) a -> p k a", p=128))
        for ch in range(NPRE):                    # stream during recurrence
            nc.sync.dma_start(
                out=fcpre[:, ch, :, :],
                in_=d_fcw[:, ch * VC:(ch + 1) * VC].rearrange(
                    "(k p) v -> p k v", p=128))

        # -------- recurrence --------
        with tc.tile_pool(name="psm", bufs=4, space="PSUM") as psm, \
             tc.tile_pool(name="psg", bufs=1, space="PSUM") as psgp, \
             tc.tile_pool(name="stp", bufs=2) as stp, \
             tc.tile_pool(name="xp", bufs=2) as xp, \
             tc.tile_pool(name="gxp", bufs=2) as gxp:
            for t in range(T):
                hprev = (lambda at: h0[:, at, :]) if t == 0 else \
                    (lambda at, _t=t: hallt[:, at, (_t - 1) * 8:(_t - 1) * 8 + 8])

                # ---- dec = 2h @ (.5 dec_W)^T ----
                pd = psm.tile([8, 512], F32, tag="sm")
                for k in range(4):
                    nc.tensor.matmul(pd[:, :], hprev(k), dwt[:, k, :],
                                     start=(k == 0), stop=(k == 3))
                dec = stp.tile([8, 512], F16, tag="dec")
                nc.vector.tensor_copy(out=dec[:, :], in_=pd[:, :])

                # ---- dec^T (+dec_b) ----
                dect = stp.tile([128, 4, 8], F16, tag="dect")
                for at in range(4):
                    ptr = psm.tile([128, 8], F16, tag="sm")
                    nc.tensor.transpose(ptr[:, :],
                                        dec[:, at * 128:(at + 1) * 128],
                                        id8[:, :])
                    nc.vector.tensor_scalar_add(
                        out=dect[:, at, :], in0=ptr[:, :],
                        scalar1=decb[:, at:at + 1])

                # ---- gates: h-part first (only needs h(t-1)) ----
                gx = gxp.tile([8, 4 * H], F16, tag="gx")
                nc.sync.dma_start(out=gx[:, :],
                                  in_=d_xp[t * 8:(t + 1) * 8, :])
                psg = psgp.tile([8, 4 * H], F32, tag="gates")
                for ch in range(4):
                    sl = slice(ch * 512, (ch + 1) * 512)
                    for k in range(4):
                        nc.tensor.matmul(psg[:, sl], hprev(k),
                                         wc[:, 4 + k, sl],
                                         start=(k == 0), stop=False)
                    nc.tensor.matmul(psg[:, sl], id8[:, :], gx[:, sl],
                                     start=False, stop=False)

                # ---- attention, pipelined over two half-batches ----
                xts = []
                for at in range(4):
                    xts.append(xp.tile([128, BN], F16, tag=f"x{at}",
                                       name=f"xt{t}_{at}"))
                ct = stp.tile([128, 4, 8], F16, tag="ct")
                for h in range(2):
                    hsl = slice(h * 4 * N, (h + 1) * 4 * N)
                    for at in range(4):
                        xt = xts[at]
                        eng = nc.vector if (at + 2 * h) % 4 < 2 else nc.gpsimd
                        eng.tensor_add(
                            out=xt[:, hsl].rearrange(
                                "p (b n) -> p b n", n=N),
                            in0=ept[:, at, hsl].rearrange(
                                "p (b n) -> p b n", n=N),
                            in1=_bcast(dect[:, at, h * 4:(h + 1) * 4], N))
                        nc.scalar.activation(out=xt[:, hsl],
                                             in_=xt[:, hsl], func=TANH)
                    psc = psm.tile([4, N], F32, tag="sm")
                    for bl in range(4):
                        b = 4 * h + bl
                        for at in range(4):
                            nc.tensor.matmul(
                                psc[:, :], ewm[:, at, bl, :],
                                xts[at][:, b * N:(b + 1) * N],
                                start=(bl == 0 and at == 0),
                                stop=(bl == 3 and at == 3))
                    atw = stp.tile([4, N], F16, tag=f"atw{h}")
                    zs = stp.tile([4, 1], F32, tag=f"zs{h}")
                    nc.scalar.activation(out=atw[:, :], in_=psc[:, :],
                                         func=EXP, accum_out=zs[:, 0:1])
                    rz = stp.tile([4, 1], F32, tag=f"rz{h}")
                    nc.vector.reciprocal(out=rz[:, :], in_=zs[:, :])

                    p1 = psm.tile([128, 4], F16, tag="sm")
                    nc.tensor.transpose(p1[:, :], atw[:, 0:128],
                                        id8[0:4, 0:4])
                    dg = atm[:, h, :, :]
                    nc.vector.tensor_copy(
                        out=bass.AP(tensor=dg.tensor, offset=dg.offset,
                                    ap=[dg.ap[0], [5, 4]]),
                        in_=p1[:, :])
                    p2 = psm.tile([128, 4], F16, tag="sm")
                    nc.tensor.transpose(p2[0:N - 128, :], atw[:, 128:N],
                                        id8[0:4, 0:4])
                    d2 = at2m[0:N - 128, h, :, :]
                    nc.vector.tensor_copy(
                        out=bass.AP(tensor=d2.tensor, offset=d2.offset,
                                    ap=[d2.ap[0], [5, 4]]),
                        in_=p2[0:N - 128, :])

                    pc = psm.tile([4, 512], F32, tag="sm")
                    for bl in range(4):
                        b = 4 * h + bl
                        nc.tensor.matmul(pc[:, :], atm[:, h, bl, :],
                                         enr[:, 2 * b, :],
                                         start=(bl == 0), stop=False)
                        nc.tensor.matmul(pc[:, :],
                                         at2m[0:N - 128, h, bl, :],
                                         enr[0:N - 128, 2 * b + 1, :],
                                         start=False, stop=(bl == 3))
                    ctxr = stp.tile([4, H], F16, tag=f"ctxr{h}")
                    nc.vector.tensor_scalar_mul(
                        out=ctxr[:, :], in0=pc[:, :], scalar1=rz[:, :])
                    for at in range(4):
                        ptr = psm.tile([128, 4], F16, tag="sm")
                        nc.tensor.transpose(
                            ptr[:, :], ctxr[:, at * 128:(at + 1) * 128],
                            id8[0:4, 0:4])
                        nc.vector.tensor_copy(
                            out=ct[:, at, h * 4:(h + 1) * 4],
                            in_=ptr[:, :])

                # ---- gates: ctx-part ----
                for ch in range(4):
                    sl = slice(ch * 512, (ch + 1) * 512)
                    for k in range(4):
                        nc.tensor.matmul(psg[:, sl], ct[:, k, :],
                                         wc[:, k, sl],
                                         start=False, stop=(k == 3))

                # ---- pointwise (i,f,o,g; sigmoid via tanh) ----
                th = stp.tile([8, 3 * H], F16, tag="th")
                nc.scalar.activation(out=th[:, :], in_=psg[:, 0:3 * H],
                                     func=TANH, scale=0.5)
                thg = stp.tile([8, H], F16, tag="thg")
                nc.scalar.activation(out=thg[:, :], in_=psg[:, 3 * H:4 * H],
                                     func=TANH)
                a2 = stp.tile([8, H], F32, tag="a2")
                nc.vector.scalar_tensor_tensor(
                    out=a2[:, :], in0=th[:, H:2 * H], scalar=1.0,
                    in1=c2[:, :], op0=ADD, op1=MULT)
                bb = stp.tile([8, H], F32, tag="bb")
                nc.vector.scalar_tensor_tensor(
                    out=bb[:, :], in0=th[:, 0:H], scalar=1.0,
                    in1=thg[:, :], op0=ADD, op1=MULT)
                nc.vector.scalar_tensor_tensor(
                    out=c2[:, :], in0=a2[:, :], scalar=0.5,
                    in1=bb[:, :], op0=MULT, op1=ADD)
                thc = stp.tile([8, H], F32, tag="thc")
                nc.scalar.activation(out=thc[:, :], in_=c2[:, :],
                                     func=TANH, scale=0.5)
                h2r = stp.tile([8, H], F16, tag="h2r")
                nc.vector.scalar_tensor_tensor(
                    out=h2r[:, :], in0=th[:, 2 * H:3 * H], scalar=1.0,
                    in1=thc[:, :], op0=ADD, op1=MULT)

                # ---- h^T into hallt[:, :, t*8:(t+1)*8] ----
                for at in range(4):
                    ptr = psm.tile([128, 8], F16, tag="sm")
                    nc.tensor.transpose(ptr[:, :],
                                        h2r[:, at * 128:(at + 1) * 128],
                                        id8[:, :])
                    nc.vector.tensor_copy(
                        out=hallt[:, at, t * 8:(t + 1) * 8], in_=ptr[:, :])

        # -------- fc --------
        with tc.tile_pool(name="fw", bufs=8) as fwp, \
             tc.tile_pool(name="fb", bufs=3) as fbp, \
             tc.tile_pool(name="fo", bufs=3) as fop, \
             tc.tile_pool(name="pf", bufs=3, space="PSUM") as pfp:
            fbt = None
            for ch in range(NCH):
                vsl = slice(ch * VC, (ch + 1) * VC)
                if ch % 8 == 0:
                    fbt = fbp.tile([1, 8, VC], F16, tag="fb",
                                   name=f"fbt{ch}")
                    nc.sync.dma_start(
                        out=fbt[:, :, :],
                        in_=d_fcb[0:1, ch * VC:(ch + 8) * VC].rearrange(
                            "p (c v) -> p c v", v=VC))
                fbc = fbt[0:1, ch % 8, :]
                if ch < NPRE:
                    fws = fcpre[:, ch, :, :]
                else:
                    fws = fwp.tile([128, 4, VC], F16, tag="fw",
                                   name=f"fw{ch}")
                    nc.sync.dma_start(
                        out=fws[:, :, :],
                        in_=d_fcw[:, vsl].rearrange(
                            "(k p) v -> p k v", p=128))
                for m in range(2):
                    mr = 128 if m == 0 else BT - 128
                    pf = pfp.tile([128, VC], F32, tag="pf")
                    for k in range(4):
                        nc.tensor.matmul(
                            pf[0:mr, :],
                            hallt[:, k, m * 128:m * 128 + mr],
                            fws[:, k, :], start=(k == 0), stop=False)
                    nc.tensor.matmul(pf[0:mr, :], ones[0:1, 0:mr],
                                     fbc, start=False, stop=True)
                    fo = fop.tile([128, VC], F32, tag="fo")
                    nc.vector.tensor_copy(out=fo[0:mr, :], in_=pf[0:mr, :])
                    nc.sync.dma_start(
                        out=d_out[m * 128:m * 128 + mr, vsl],
                        in_=fo[0:mr, :])


_PROGRAM = None


def kernel(**inputs) -> np.ndarray:
    global _PROGRAM
    if _PROGRAM is None:
        _PROGRAM = build_program()
    in_maps = [prep_core(c, inputs) for c in range(NC)]
    res = run_bass_kernel_spmd(_PROGRAM, in_maps, core_ids=list(range(NC)))
    out = np.zeros((B, L, V), np.float32)
    for c in range(NC):
        lg = res.results[c]["logits"].reshape(T, BS, V)
        out[c * BS:(c + 1) * BS, 1:, :] = lg.transpose(1, 0, 2)
    return out


if __name__ == "__main__":
    import reference
    ins = {k: np.asarray(v) for k, v in reference.setup_inputs().items()}
    got = kernel(**ins)
    exp = np.asarray(reference.reference(**reference.setup_inputs()))
    err = np.abs(got - exp).max() / (np.abs(exp).max() + 1e-12)
    print("Relative error:", err)


# revision 16
# speedup vs baseline: 1.1340x; 1.0159x over previous
"""DecoderLSTM (Bahdanau attention + LSTM + vocab fc) on 8 Trainium2 cores.

Sharding: data-parallel over batch (64 -> 8 rows/core); the sequential scan
stays local per core; zero collectives. Host only shards/casts/transposes
inputs and reassembles the output.

Per-core (b=8 rows, N=196, H=E=A=512, V=20000, T=19 steps):
  setup:  X_proj[t*8+b,:] = [emb(x);1] @ [W_ih_x;b_ih+b_hh]^T -> DRAM scratch
          enc_proj^T[a,(b,n)] = enc_W @ enc^T (+enc_b)        -> SBUF resident
  step t: dec = 2h @ (.5 dec_W)^T; dec^T via PE transpose (+dec_b)
          X = tanh(enc_proj^T + bcast dec^T)   [128,1568] x4  (DVE/GPSIMD+ACT)
          scores: M=8-redundant matmul w_e . X; softmax via exp (fused row
          sums) on the valid diagonal rows; attw^T via PE transpose
          ctx_b = attw_b . enc_b (M=8 redundant, row b valid) -> ctx^T
          gates = [ctx;2h] @ Wc^T + X_proj[t]  (identity-matmul accumulate)
          pointwise with sigmoid(x)=(tanh(x/2)+1)/2; states C2=2c, H2=2h
          (factor 2 folded into host-prescaled 0.5*{W_hh, dec_W, fc_W})
  fc:     logits = (H2_all)^T @ (.5 fc_W)^T + fc_b, 500-col vocab chunks
"""

import numpy as np

import concourse.bass as bass
import concourse.bacc as bacc
import concourse.tile as tile
from concourse import mybir
from concourse.bass_utils import run_bass_kernel_spmd

F16 = mybir.dt.float16
F32 = mybir.dt.float32

B, N, H, E, A, V, L = 64, 196, 512, 512, 512, 20000, 20
T = L - 1            # 19 decode steps
NC = 8               # cores
BS = B // NC         # 8 batch rows per core
BN = BS * N          # 1568
BT = T * BS          # 152 rows, t-major (row = t*8 + b)
VC = 500             # fc vocab chunk width
NCH = V // VC        # 40

# gate reorder [i,f,g,o] -> [i,f,o,g] so tanh(0.5*x) covers cols 0:1536
PERM = np.concatenate([np.arange(0, H), np.arange(H, 2 * H),
                       np.arange(3 * H, 4 * H), np.arange(2 * H, 3 * H)])

TANH = mybir.ActivationFunctionType.Tanh
EXP = mybir.ActivationFunctionType.Exp
ADD = mybir.AluOpType.add
MULT = mybir.AluOpType.mult


def prep_core(core, inputs):
    """Per-core numpy input dict (shard + transpose + cast only)."""
    f32 = np.float32
    bsl = slice(core * BS, (core + 1) * BS)
    enc = np.asarray(inputs["encoder_outputs"][bsl], f32)      # [8,196,512]

    enc_t = np.ascontiguousarray(enc.reshape(BN, H).T).astype(np.float16)
    enc_r = np.zeros((2 * BS, 128, H), np.float16)
    for b in range(BS):
        enc_r[2 * b, :128] = enc[b, :128]
        enc_r[2 * b + 1, :N - 128] = enc[b, 128:]

    caps = np.asarray(inputs["captions"][bsl])[:, :T]          # [8,19]
    es = np.asarray(inputs["emb"], f32)[caps]                  # [8,19,512]
    emb_flat = es.transpose(1, 0, 2).reshape(BT, E)            # t-major rows
    emb_cat = np.concatenate(
        [emb_flat.T, np.ones((1, BT), f32)], 0).astype(np.float16)

    wih = np.asarray(inputs["W_ih"], f32)[PERM]                # [2048,1024]
    whh = np.asarray(inputs["W_hh"], f32)[PERM]
    bias = (np.asarray(inputs["b_ih"], f32) +
            np.asarray(inputs["b_hh"], f32))[PERM]
    wihxb_t = np.concatenate(
        [wih[:, :E].T, bias[None, :]], 0).astype(np.float16)   # [513,2048]
    wc_t = np.concatenate(
        [wih[:, E:].T, 0.5 * whh.T], 0).astype(np.float16)     # [1024,2048]

    dec_wt = (0.5 * np.asarray(inputs["dec_W"], f32).T).astype(np.float16)
    enc_wt = np.ascontiguousarray(
        np.asarray(inputs["enc_W"], f32).T).astype(np.float16)  # [H,A]
    decb = np.ascontiguousarray(
        np.asarray(inputs["dec_b"], f32).reshape(4, 128).T)     # [128,4]
    encb = np.ascontiguousarray(
        np.asarray(inputs["enc_b"], f32).reshape(4, 128).T)
    ew = np.ascontiguousarray(
        np.asarray(inputs["energy_W"], f32)[0].reshape(4, 128).T
    ).astype(np.float16)                                        # [128,4]
    fcw_t = np.ascontiguousarray(
        0.5 * np.asarray(inputs["fc_W"], f32).T).astype(np.float16)
    fcb = np.ascontiguousarray(
        np.asarray(inputs["fc_b"], f32)[None, :]).astype(np.float16)
    id8 = np.eye(8, dtype=np.float16)

    return {"enc_t": enc_t, "enc_r": enc_r, "emb_cat": emb_cat,
            "wihxb_t": wihxb_t, "wc_t": wc_t, "dec_wt": dec_wt,
            "enc_wt": enc_wt, "decb": decb, "encb": encb, "ew": ew,
            "fcw_t": fcw_t, "fcb": fcb, "id8": id8}


def _bcast(ap, n):
    """Append an innermost step-0 (broadcast) dim of size n to an AP."""
    return bass.AP(tensor=ap.tensor, offset=ap.offset,
                   ap=list(ap.ap) + [[0, n]])


def build_program():
    nc = bacc.Bacc("TRN2", target_bir_lowering=False, debug=False,
                   num_devices=NC)
    d_enc_t = nc.dram_tensor("enc_t", [H, BN], F16, kind="ExternalInput")
    d_enc_r = nc.dram_tensor("enc_r", [2 * BS, 128, H], F16,
                             kind="ExternalInput")
    d_emb = nc.dram_tensor("emb_cat", [E + 1, BT], F16, kind="ExternalInput")
    d_wx = nc.dram_tensor("wihxb_t", [E + 1, 4 * H], F16,
                          kind="ExternalInput")
    d_wc = nc.dram_tensor("wc_t", [2 * H, 4 * H], F16, kind="ExternalInput")
    d_dwt = nc.dram_tensor("dec_wt", [H, A], F16, kind="ExternalInput")
    d_ewt = nc.dram_tensor("enc_wt", [H, A], F16, kind="ExternalInput")
    d_decb = nc.dram_tensor("decb", [128, 4], F32, kind="ExternalInput")
    d_encb = nc.dram_tensor("encb", [128, 4], F32, kind="ExternalInput")
    d_ew = nc.dram_tensor("ew", [128, 4], F16, kind="ExternalInput")
    d_fcw = nc.dram_tensor("fcw_t", [H, V], F16, kind="ExternalInput")
    d_fcb = nc.dram_tensor("fcb", [1, V], F16, kind="ExternalInput")
    d_id8 = nc.dram_tensor("id8", [8, 8], F16, kind="ExternalInput")
    d_out = nc.dram_tensor("logits", [BT, V], F32, kind="ExternalOutput")
    d_xp = nc.dram_tensor("xproj", [BT, 4 * H], F16, kind="Internal")

    with tile.TileContext(nc) as tc:
        _build_body(nc, tc, d_enc_t, d_enc_r, d_emb, d_wx, d_wc, d_dwt,
                    d_ewt, d_decb, d_encb, d_ew, d_fcw, d_fcb, d_id8,
                    d_out, d_xp)
    nc.compile()
    return nc


def _build_body(nc, tc, d_enc_t, d_enc_r, d_emb, d_wx, d_wc, d_dwt, d_ewt,
                d_decb, d_encb, d_ew, d_fcw, d_fcb, d_id8, d_out, d_xp):
    with tc.tile_pool(name="res", bufs=1) as res:
        # -------- residents --------
        ept = res.tile([128, 4, BN], F16)        # enc_proj^T a-tiles
        wc = res.tile([128, 8, 4 * H], F16)
        enr = res.tile([128, 2 * BS, H], F16)
        dwt = res.tile([128, 4, A], F16)
        decb = res.tile([128, 4], F32)
        encb = res.tile([128, 4], F32)
        ewm = res.tile([128, 4, 4, 4], F16)    # diag: [:,at,bl,bl]=ew
        atm = res.tile([128, 2, 4, 4], F16)    # per-half diag attw (n<128)
        at2m = res.tile([128, 2, 4, 4], F16)   # per-half diag (n=128:196)
        id8 = res.tile([8, 8], F16)
        hallt = res.tile([128, 4, BT], F16)      # H2^T, all steps
        h0 = res.tile([128, 4, 8], F16)
        c2 = res.tile([8, H], F32)
        ones = res.tile([1, 128], F16)
        NPRE = 19                                 # prefetched fc chunks
        fcpre = res.tile([128, NPRE, 4, VC], F16)

        nc.sync.dma_start(out=decb[:, :], in_=d_decb[:, :])
        nc.sync.dma_start(out=encb[:, :], in_=d_encb[:, :])
        nc.sync.dma_start(out=id8[:, :], in_=d_id8[:, :])
        ew_col = res.tile([128, 4], F16)
        nc.sync.dma_start(out=ew_col[:, :], in_=d_ew[:, :])
        nc.vector.memset(ewm[:, :, :, :], 0.0)
        nc.vector.memset(atm[:, :, :, :], 0.0)
        nc.vector.memset(at2m[:, :, :, :], 0.0)
        for at in range(4):
            col = ew_col[:, at:at + 1]
            dg = ewm[:, at, :, :]
            nc.vector.tensor_copy(
                out=bass.AP(tensor=dg.tensor, offset=dg.offset,
                            ap=[dg.ap[0], [5, 4]]),
                in_=bass.AP(tensor=col.tensor, offset=col.offset,
                            ap=[col.ap[0], [0, 4]]))
        nc.vector.memset(h0[:, :, :], 0.0)
        nc.vector.memset(c2[:, :], 0.0)
        nc.vector.memset(ones[:, :], 1.0)

        # -------- setup: X_proj to DRAM scratch --------
        with tc.tile_pool(name="sx", bufs=1) as sx, \
             tc.tile_pool(name="sxp", bufs=2, space="PSUM") as sxp, \
             tc.tile_pool(name="sxs", bufs=3) as sxs:
            ec = sx.tile([128, 5, BT], F16)
            wx = sx.tile([128, 5, 4 * H], F16)
            nc.sync.dma_start(
                out=ec[:, 0:4, :],
                in_=d_emb[0:512, :].rearrange("(k p) t -> p k t", p=128))
            nc.sync.dma_start(
                out=wx[:, 0:4, :],
                in_=d_wx[0:512, :].rearrange("(k p) g -> p k g", p=128))
            nc.sync.dma_start(out=ec[0:1, 4, :], in_=d_emb[512:513, :])
            nc.sync.dma_start(out=wx[0:1, 4, :], in_=d_wx[512:513, :])
            for m in range(2):
                mr = 128 if m == 0 else BT - 128
                for ch in range(4):
                    pt = sxp.tile([128, 512], F32, tag="sxp")
                    for k in range(5):
                        kr = 128 if k < 4 else 1
                        nc.tensor.matmul(
                            pt[0:mr, :],
                            ec[0:kr, k, m * 128:m * 128 + mr],
                            wx[0:kr, k, ch * 512:(ch + 1) * 512],
                            start=(k == 0), stop=(k == 4))
                    st = sxs.tile([128, 512], F16, tag="st")
                    nc.vector.tensor_copy(out=st[0:mr, :], in_=pt[0:mr, :])
                    nc.sync.dma_start(
                        out=d_xp[m * 128:m * 128 + mr,
                                 ch * 512:(ch + 1) * 512],
                        in_=st[0:mr, :])

        # -------- setup: enc_proj^T (+enc_b) --------
        with tc.tile_pool(name="se", bufs=1) as se, \
             tc.tile_pool(name="sep", bufs=2, space="PSUM") as sep:
            et = se.tile([128, 4, BN], F16)
            ewt = se.tile([128, 4, A], F16)
            nc.sync.dma_start(
                out=et[:, :, :],
                in_=d_enc_t[:, :].rearrange("(k p) n -> p k n", p=128))
            nc.sync.dma_start(
                out=ewt[:, :, :],
                in_=d_ewt[:, :].rearrange("(k p) a -> p k a", p=128))
            for at in range(4):                      # a-tile = out m-tile
                for ch in range(4):                  # 1568 = 4*392
                    pt = sep.tile([128, 392], F32, tag="sep")
                    for k in range(4):
                        nc.tensor.matmul(
                            pt[:, :],
                            ewt[:, k, at * 128:(at + 1) * 128],
                            et[:, k, ch * 392:(ch + 1) * 392],
                            start=(k == 0), stop=(k == 3))
                    nc.vector.tensor_scalar_add(
                        out=ept[:, at, ch * 392:(ch + 1) * 392],
                        in0=pt[:, :], scalar1=encb[:, at:at + 1])

        # load remaining residents (batched DMAs)
        nc.sync.dma_start(out=wc[:, :, :],
                          in_=d_wc[:, :].rearrange("(k p) g -> p k g", p=128))
        nc.sync.dma_start(out=enr[:, :, :],
                          in_=d_enc_r[:, :, :].rearrange("j p h -> p j h"))
        nc.sync.dma_start(out=dwt[:, :, :],
                          in_=d_dwt[:, :].rearrange("(k p) a -> p k a", p=128))
        for ch in range(NPRE):                    # stream during recurrence
            nc.sync.dma_start(
                out=fcpre[:, ch, :, :],
                in_=d_fcw[:, ch * VC:(ch + 1) * VC].rearrange(
                    "(k p) v -> p k v", p=128))

        # -------- recurrence --------
        with tc.tile_pool(name="psm", bufs=4, space="PSUM") as psm, \
             tc.tile_pool(name="psg", bufs=1, space="PSUM") as psgp, \
             tc.tile_pool(name="stp", bufs=2) as stp, \
             tc.tile_pool(name="xp", bufs=2) as xp, \
             tc.tile_pool(name="gxp", bufs=2) as gxp:
            for t in range(T):
                hprev = (lambda at: h0[:, at, :]) if t == 0 else \
                    (lambda at, _t=t: hallt[:, at, (_t - 1) * 8:(_t - 1) * 8 + 8])

                # ---- dec = 2h @ (.5 dec_W)^T ----
                pd = psm.tile([8, 512], F32, tag="sm")
                for k in range(4):
                    nc.tensor.matmul(pd[:, :], hprev(k), dwt[:, k, :],
                                     start=(k == 0), stop=(k == 3))
                dec = stp.tile([8, 512], F16, tag="dec")
                nc.vector.tensor_copy(out=dec[:, :], in_=pd[:, :])

                # ---- dec^T (+dec_b) ----
                dect = stp.tile([128, 4, 8], F16, tag="dect")
                for at in range(4):
                    ptr = psm.tile([128, 8], F16, tag="sm")
                    nc.tensor.transpose(ptr[:, :],
                                        dec[:, at * 128:(at + 1) * 128],
                                        id8[:, :])
                    nc.vector.tensor_scalar_add(
                        out=dect[:, at, :], in0=ptr[:, :],
                        scalar1=decb[:, at:at + 1])

                # ---- gates: h-part first (only needs h(t-1)) ----
                gx = gxp.tile([8, 4 * H], F16, tag="gx")
                nc.sync.dma_start(out=gx[:, :],
                                  in_=d_xp[t * 8:(t + 1) * 8, :])
                psg = psgp.tile([8, 4 * H], F32, tag="gates")
                for ch in range(4):
                    sl = slice(ch * 512, (ch + 1) * 512)
                    for k in range(4):
                        nc.tensor.matmul(psg[:, sl], hprev(k),
                                         wc[:, 4 + k, sl],
                                         start=(k == 0), stop=False)
                    nc.tensor.matmul(psg[:, sl], id8[:, :], gx[:, sl],
                                     start=False, stop=False)

                # ---- attention, pipelined over two half-batches ----
                xts = []
                for at in range(4):
                    xts.append(xp.tile([128, BN], F16, tag=f"x{at}",
                                       name=f"xt{t}_{at}"))
                ct = stp.tile([128, 4, 8], F16, tag="ct")
                for h in range(2):
                    hsl = slice(h * 4 * N, (h + 1) * 4 * N)
                    for at in range(4):
                        xt = xts[at]
                        eng = (nc.gpsimd if at == (3 if h == 0 else 1)
                               else nc.vector)
                        eng.tensor_add(
                            out=xt[:, hsl].rearrange(
                                "p (b n) -> p b n", n=N),
                            in0=ept[:, at, hsl].rearrange(
                                "p (b n) -> p b n", n=N),
                            in1=_bcast(dect[:, at, h * 4:(h + 1) * 4], N))
                        nc.scalar.activation(out=xt[:, hsl],
                                             in_=xt[:, hsl], func=TANH)
                    psc = psm.tile([4, N], F32, tag="sm")
                    for bl in range(4):
                        b = 4 * h + bl
                        for at in range(4):
                            nc.tensor.matmul(
                                psc[:, :], ewm[:, at, bl, :],
                                xts[at][:, b * N:(b + 1) * N],
                                start=(bl == 0 and at == 0),
                                stop=(bl == 3 and at == 3))
                    atw = stp.tile([4, N], F16, tag=f"atw{h}")
                    zs = stp.tile([4, 1], F32, tag=f"zs{h}")
                    nc.scalar.activation(out=atw[:, :], in_=psc[:, :],
                                         func=EXP, accum_out=zs[:, 0:1])
                    rz = stp.tile([4, 1], F32, tag=f"rz{h}")
                    nc.vector.reciprocal(out=rz[:, :], in_=zs[:, :])

                    p1 = psm.tile([128, 4], F16, tag="sm")
                    nc.tensor.transpose(p1[:, :], atw[:, 0:128],
                                        id8[0:4, 0:4])
                    dg = atm[:, h, :, :]
                    nc.vector.tensor_copy(
                        out=bass.AP(tensor=dg.tensor, offset=dg.offset,
                                    ap=[dg.ap[0], [5, 4]]),
                        in_=p1[:, :])
                    p2 = psm.tile([128, 4], F16, tag="sm")
                    nc.tensor.transpose(p2[0:N - 128, :], atw[:, 128:N],
                                        id8[0:4, 0:4])
                    d2 = at2m[0:N - 128, h, :, :]
                    nc.vector.tensor_copy(
                        out=bass.AP(tensor=d2.tensor, offset=d2.offset,
                                    ap=[d2.ap[0], [5, 4]]),
                        in_=p2[0:N - 128, :])

                    pc = psm.tile([4, 512], F32, tag="sm")
                    for bl in range(4):
                        b = 4 * h + bl
                        nc.tensor.matmul(pc[:, :], atm[:, h, bl, :],
                                         enr[:, 2 * b, :],
                                         start=(bl == 0), stop=False)
                        nc.tensor.matmul(pc[:, :],
                                         at2m[0:N - 128, h, bl, :],
                                         enr[0:N - 128, 2 * b + 1, :],
                                         start=False, stop=(bl == 3))
                    ctxr = stp.tile([4, H], F16, tag=f"ctxr{h}")
                    nc.vector.tensor_scalar_mul(
                        out=ctxr[:, :], in0=pc[:, :], scalar1=rz[:, :])
                    for at in range(4):
                        ptr = psm.tile([128, 4], F16, tag="sm")
                        nc.tensor.transpose(
                            ptr[:, :], ctxr[:, at * 128:(at + 1) * 128],
                            id8[0:4, 0:4])
                        nc.vector.tensor_copy(
                            out=ct[:, at, h * 4:(h + 1) * 4],
                            in_=ptr[:, :])

                # ---- gates: ctx-part ----
                for ch in range(4):
                    sl = slice(ch * 512, (ch + 1) * 512)
                    for k in range(4):
                        nc.tensor.matmul(psg[:, sl], ct[:, k, :],
                                         wc[:, k, sl],
                                         start=False, stop=(k == 3))

                # ---- pointwise (i,f,o,g; sigmoid via tanh) ----
                th = stp.tile([8, 3 * H], F16, tag="th")
                nc.scalar.activation(out=th[:, :], in_=psg[:, 0:3 * H],
                                     func=TANH, scale=0.5)
                thg = stp.tile([8, H], F16, tag="thg")
                nc.scalar.activation(out=thg[:, :], in_=psg[:, 3 * H:4 * H],
                                     func=TANH)
                a2 = stp.tile([8, H], F32, tag="a2")
                nc.vector.scalar_tensor_tensor(
                    out=a2[:, :], in0=th[:, H:2 * H], scalar=1.0,
                    in1=c2[:, :], op0=ADD, op1=MULT)
                bb = stp.tile([8, H], F32, tag="bb")
                nc.vector.scalar_tensor_tensor(
                    out=bb[:, :], in0=th[:, 0:H], scalar=1.0,
                    in1=thg[:, :], op0=ADD, op1=MULT)
                nc.vector.scalar_tensor_tensor(
                    out=c2[:, :], in0=a2[:, :], scalar=0.5,
                    in1=bb[:, :], op0=MULT, op1=ADD)
                thc = stp.tile([8, H], F32, tag="thc")
                nc.scalar.activation(out=thc[:, :], in_=c2[:, :],
                                     func=TANH, scale=0.5)
                h2r = stp.tile([8, H], F16, tag="h2r")
                nc.vector.scalar_tensor_tensor(
                    out=h2r[:, :], in0=th[:, 2 * H:3 * H], scalar=1.0,
                    in1=thc[:, :], op0=ADD, op1=MULT)

                # ---- h^T into hallt[:, :, t*8:(t+1)*8] ----
                for at in range(4):
                    ptr = psm.tile([128, 8], F16, tag="sm")
                    nc.tensor.transpose(ptr[:, :],
                                        h2r[:, at * 128:(at + 1) * 128],
                                        id8[:, :])
                    nc.vector.tensor_copy(
                        out=hallt[:, at, t * 8:(t + 1) * 8], in_=ptr[:, :])

        # -------- fc --------
        with tc.tile_pool(name="fw", bufs=8) as fwp, \
             tc.tile_pool(name="fb", bufs=3) as fbp, \
             tc.tile_pool(name="fo", bufs=3) as fop, \
             tc.tile_pool(name="pf", bufs=3, space="PSUM") as pfp:
            fbt = None
            for ch in range(NCH):
                vsl = slice(ch * VC, (ch + 1) * VC)
                if ch % 8 == 0:
                    fbt = fbp.tile([1, 8, VC], F16, tag="fb",
                                   name=f"fbt{ch}")
                    nc.sync.dma_start(
                        out=fbt[:, :, :],
                        in_=d_fcb[0:1, ch * VC:(ch + 8) * VC].rearrange(
                            "p (c v) -> p c v", v=VC))
                fbc = fbt[0:1, ch % 8, :]
                if ch < NPRE:
                    fws = fcpre[:, ch, :, :]
                else:
                    fws = fwp.tile([128, 4, VC], F16, tag="fw",
                                   name=f"fw{ch}")
                    nc.sync.dma_start(
                        out=fws[:, :, :],
                        in_=d_fcw[:, vsl].rearrange(
                            "(k p) v -> p k v", p=128))
                for m in range(2):
                    mr = 128 if m == 0 else BT - 128
                    pf = pfp.tile([128, VC], F32, tag="pf")
                    for k in range(4):
                        nc.tensor.matmul(
                            pf[0:mr, :],
                            hallt[:, k, m * 128:m * 128 + mr],
                            fws[:, k, :], start=(k == 0), stop=False)
                    nc.tensor.matmul(pf[0:mr, :], ones[0:1, 0:mr],
                                     fbc, start=False, stop=True)
                    fo = fop.tile([128, VC], F32, tag="fo")
                    nc.vector.tensor_copy(out=fo[0:mr, :], in_=pf[0:mr, :])
                    nc.sync.dma_start(
                        out=d_out[m * 128:m * 128 + mr, vsl],
                        in_=fo[0:mr, :])


_PROGRAM = None


def kernel(**inputs) -> np.ndarray:
    global _PROGRAM
    if _PROGRAM is None:
        _PROGRAM = build_program()
    in_maps = [prep_core(c, inputs) for c in range(NC)]
    res = run_bass_kernel_spmd(_PROGRAM, in_maps, core_ids=list(range(NC)))
    out = np.zeros((B, L, V), np.float32)
    for c in range(NC):
        lg = res.results[c]["logits"].reshape(T, BS, V)
        out[c * BS:(c + 1) * BS, 1:, :] = lg.transpose(1, 0, 2)
    return out


if __name__ == "__main__":
    import reference
    ins = {k: np.asarray(v) for k, v in reference.setup_inputs().items()}
    got = kernel(**ins)
    exp = np.asarray(reference.reference(**reference.setup_inputs()))
    err = np.abs(got - exp).max() / (np.abs(exp).max() + 1e-12)
    print("Relative error:", err)


# revision 20
# speedup vs baseline: 1.1770x; 1.0379x over previous
"""DecoderLSTM (Bahdanau attention + LSTM + vocab fc) on 8 Trainium2 cores.

Sharding: data-parallel over batch (64 -> 8 rows/core); the sequential scan
stays local per core; zero collectives. Host only shards/casts/transposes
inputs and reassembles the output.

Per-core (b=8 rows, N=196, H=E=A=512, V=20000, T=19 steps):
  setup:  X_proj[t*8+b,:] = [emb(x);1] @ [W_ih_x;b_ih+b_hh]^T -> DRAM scratch
          enc_proj^T[a,(b,n)] = enc_W @ enc^T (+enc_b)        -> SBUF resident
  step t: dec = 2h @ (.5 dec_W)^T; dec^T via PE transpose (+dec_b)
          X = tanh(enc_proj^T + bcast dec^T)   [128,1568] x4  (DVE/GPSIMD+ACT)
          scores: M=8-redundant matmul w_e . X; softmax via exp (fused row
          sums) on the valid diagonal rows; attw^T via PE transpose
          ctx_b = attw_b . enc_b (M=8 redundant, row b valid) -> ctx^T
          gates = [ctx;2h] @ Wc^T + X_proj[t]  (identity-matmul accumulate)
          pointwise with sigmoid(x)=(tanh(x/2)+1)/2; states C2=2c, H2=2h
          (factor 2 folded into host-prescaled 0.5*{W_hh, dec_W, fc_W})
  fc:     logits = (H2_all)^T @ (.5 fc_W)^T + fc_b, 500-col vocab chunks
"""

import numpy as np

import concourse.bass as bass
import concourse.bacc as bacc
import concourse.tile as tile
from concourse import mybir
from concourse.bass_utils import run_bass_kernel_spmd

F16 = mybir.dt.float16
F32 = mybir.dt.float32

B, N, H, E, A, V, L = 64, 196, 512, 512, 512, 20000, 20
T = L - 1            # 19 decode steps
NC = 8               # cores
BS = B // NC         # 8 batch rows per core
BN = BS * N          # 1568
BT = T * BS          # 152 rows, t-major (row = t*8 + b)
VC = 500             # fc vocab chunk width
NCH = V // VC        # 40

# gate reorder [i,f,g,o] -> [i,f,o,g] so tanh(0.5*x) covers cols 0:1536
PERM = np.concatenate([np.arange(0, H), np.arange(H, 2 * H),
                       np.arange(3 * H, 4 * H), np.arange(2 * H, 3 * H)])

TANH = mybir.ActivationFunctionType.Tanh
EXP = mybir.ActivationFunctionType.Exp
ADD = mybir.AluOpType.add
MULT = mybir.AluOpType.mult


def prep_core(core, inputs):
    """Per-core numpy input dict (shard + transpose + cast only)."""
    f32 = np.float32
    bsl = slice(core * BS, (core + 1) * BS)
    enc = np.asarray(inputs["encoder_outputs"][bsl], f32)      # [8,196,512]

    enc_t = np.ascontiguousarray(enc.reshape(BN, H).T).astype(np.float16)
    enc_r = np.zeros((2 * BS, 128, H), np.float16)
    for b in range(BS):
        enc_r[2 * b, :128] = enc[b, :128]
        enc_r[2 * b + 1, :N - 128] = enc[b, 128:]

    caps = np.asarray(inputs["captions"][bsl])[:, :T]          # [8,19]
    es = np.asarray(inputs["emb"], f32)[caps]                  # [8,19,512]
    emb_flat = es.transpose(1, 0, 2).reshape(BT, E)            # t-major rows
    emb_cat = np.concatenate(
        [emb_flat.T, np.ones((1, BT), f32)], 0).astype(np.float16)

    wih = np.asarray(inputs["W_ih"], f32)[PERM]                # [2048,1024]
    whh = np.asarray(inputs["W_hh"], f32)[PERM]
    bias = (np.asarray(inputs["b_ih"], f32) +
            np.asarray(inputs["b_hh"], f32))[PERM]
    wihxb_t = np.concatenate(
        [wih[:, :E].T, bias[None, :]], 0).astype(np.float16)   # [513,2048]
    wc_t = np.concatenate(
        [wih[:, E:].T, 0.5 * whh.T], 0).astype(np.float16)     # [1024,2048]

    dec_wt = (0.5 * np.asarray(inputs["dec_W"], f32).T).astype(np.float16)
    enc_wt = np.ascontiguousarray(
        np.asarray(inputs["enc_W"], f32).T).astype(np.float16)  # [H,A]
    decb = np.ascontiguousarray(
        np.asarray(inputs["dec_b"], f32).reshape(4, 128).T)     # [128,4]
    encb = np.ascontiguousarray(
        np.asarray(inputs["enc_b"], f32).reshape(4, 128).T)
    ew = np.ascontiguousarray(
        np.asarray(inputs["energy_W"], f32)[0].reshape(4, 128).T
    ).astype(np.float16)                                        # [128,4]
    fcw_t = np.ascontiguousarray(
        0.5 * np.asarray(inputs["fc_W"], f32).T).astype(np.float16)
    fcb = np.ascontiguousarray(
        np.asarray(inputs["fc_b"], f32)[None, :]).astype(np.float16)
    id8 = np.eye(8, dtype=np.float16)

    return {"enc_t": enc_t, "enc_r": enc_r, "emb_cat": emb_cat,
            "wihxb_t": wihxb_t, "wc_t": wc_t, "dec_wt": dec_wt,
            "enc_wt": enc_wt, "decb": decb, "encb": encb, "ew": ew,
            "fcw_t": fcw_t, "fcb": fcb, "id8": id8}


def _bcast(ap, n):
    """Append an innermost step-0 (broadcast) dim of size n to an AP."""
    return bass.AP(tensor=ap.tensor, offset=ap.offset,
                   ap=list(ap.ap) + [[0, n]])


def build_program():
    nc = bacc.Bacc("TRN2", target_bir_lowering=False, debug=False,
                   num_devices=NC)
    d_enc_t = nc.dram_tensor("enc_t", [H, BN], F16, kind="ExternalInput")
    d_enc_r = nc.dram_tensor("enc_r", [2 * BS, 128, H], F16,
                             kind="ExternalInput")
    d_emb = nc.dram_tensor("emb_cat", [E + 1, BT], F16, kind="ExternalInput")
    d_wx = nc.dram_tensor("wihxb_t", [E + 1, 4 * H], F16,
                          kind="ExternalInput")
    d_wc = nc.dram_tensor("wc_t", [2 * H, 4 * H], F16, kind="ExternalInput")
    d_dwt = nc.dram_tensor("dec_wt", [H, A], F16, kind="ExternalInput")
    d_ewt = nc.dram_tensor("enc_wt", [H, A], F16, kind="ExternalInput")
    d_decb = nc.dram_tensor("decb", [128, 4], F32, kind="ExternalInput")
    d_encb = nc.dram_tensor("encb", [128, 4], F32, kind="ExternalInput")
    d_ew = nc.dram_tensor("ew", [128, 4], F16, kind="ExternalInput")
    d_fcw = nc.dram_tensor("fcw_t", [H, V], F16, kind="ExternalInput")
    d_fcb = nc.dram_tensor("fcb", [1, V], F16, kind="ExternalInput")
    d_id8 = nc.dram_tensor("id8", [8, 8], F16, kind="ExternalInput")
    d_out = nc.dram_tensor("logits", [BT, V], F32, kind="ExternalOutput")
    d_xp = nc.dram_tensor("xproj", [BT, 4 * H], F16, kind="Internal")

    with tile.TileContext(nc) as tc:
        _build_body(nc, tc, d_enc_t, d_enc_r, d_emb, d_wx, d_wc, d_dwt,
                    d_ewt, d_decb, d_encb, d_ew, d_fcw, d_fcb, d_id8,
                    d_out, d_xp)
    nc.compile()
    return nc


def _build_body(nc, tc, d_enc_t, d_enc_r, d_emb, d_wx, d_wc, d_dwt, d_ewt,
                d_decb, d_encb, d_ew, d_fcw, d_fcb, d_id8, d_out, d_xp):
    with tc.tile_pool(name="res", bufs=1) as res:
        # -------- residents --------
        ept = res.tile([128, 4, BN], F16)        # enc_proj^T a-tiles
        wc = res.tile([128, 8, 4 * H], F16)
        enr = res.tile([128, 2 * BS, H], F16)
        dwt = res.tile([128, 4, A], F16)
        decb = res.tile([128, 4], F32)
        encb = res.tile([128, 4], F32)
        ewm = res.tile([128, 4, 4, 4], F16)    # diag: [:,at,bl,bl]=ew
        atm = res.tile([128, 2, 4, 4], F16)    # per-half diag attw (n<128)
        at2m = res.tile([128, 2, 4, 4], F16)   # per-half diag (n=128:196)
        id8 = res.tile([8, 8], F16)
        hallt = res.tile([128, 4, BT], F16)      # H2^T, all steps
        h0 = res.tile([128, 4, 8], F16)
        c2 = res.tile([8, H], F32)
        ones = res.tile([1, 128], F16)
        NPRE = 21                                 # prefetched fc chunks
        fcpre = res.tile([128, NPRE, 4, VC], F16)

        nc.sync.dma_start(out=decb[:, :], in_=d_decb[:, :])
        nc.sync.dma_start(out=encb[:, :], in_=d_encb[:, :])
        nc.sync.dma_start(out=id8[:, :], in_=d_id8[:, :])
        ew_col = res.tile([128, 4], F16)
        nc.sync.dma_start(out=ew_col[:, :], in_=d_ew[:, :])
        nc.vector.memset(ewm[:, :, :, :], 0.0)
        nc.vector.memset(atm[:, :, :, :], 0.0)
        nc.vector.memset(at2m[:, :, :, :], 0.0)
        for at in range(4):
            col = ew_col[:, at:at + 1]
            dg = ewm[:, at, :, :]
            nc.vector.tensor_copy(
                out=bass.AP(tensor=dg.tensor, offset=dg.offset,
                            ap=[dg.ap[0], [5, 4]]),
                in_=bass.AP(tensor=col.tensor, offset=col.offset,
                            ap=[col.ap[0], [0, 4]]))
        nc.vector.memset(h0[:, :, :], 0.0)
        nc.vector.memset(c2[:, :], 0.0)
        nc.vector.memset(ones[:, :], 1.0)

        # -------- setup: X_proj to DRAM scratch --------
        with tc.tile_pool(name="sx", bufs=1) as sx, \
             tc.tile_pool(name="sxp", bufs=2, space="PSUM") as sxp, \
             tc.tile_pool(name="sxs", bufs=3) as sxs:
            ec = sx.tile([128, 5, BT], F16)
            wx = sx.tile([128, 5, 4 * H], F16)
            nc.sync.dma_start(
                out=ec[:, 0:4, :],
                in_=d_emb[0:512, :].rearrange("(k p) t -> p k t", p=128))
            nc.sync.dma_start(
                out=wx[:, 0:4, :],
                in_=d_wx[0:512, :].rearrange("(k p) g -> p k g", p=128))
            nc.sync.dma_start(out=ec[0:1, 4, :], in_=d_emb[512:513, :])
            nc.sync.dma_start(out=wx[0:1, 4, :], in_=d_wx[512:513, :])
            for m in range(2):
                mr = 128 if m == 0 else BT - 128
                for ch in range(4):
                    pt = sxp.tile([128, 512], F32, tag="sxp")
                    for k in range(5):
                        kr = 128 if k < 4 else 1
                        nc.tensor.matmul(
                            pt[0:mr, :],
                            ec[0:kr, k, m * 128:m * 128 + mr],
                            wx[0:kr, k, ch * 512:(ch + 1) * 512],
                            start=(k == 0), stop=(k == 4))
                    st = sxs.tile([128, 512], F16, tag="st")
                    nc.vector.tensor_copy(out=st[0:mr, :], in_=pt[0:mr, :])
                    nc.sync.dma_start(
                        out=d_xp[m * 128:m * 128 + mr,
                                 ch * 512:(ch + 1) * 512],
                        in_=st[0:mr, :])

        # -------- setup: enc_proj^T (+enc_b) --------
        with tc.tile_pool(name="se", bufs=1) as se, \
             tc.tile_pool(name="sep", bufs=2, space="PSUM") as sep:
            et = se.tile([128, 4, BN], F16)
            ewt = se.tile([128, 4, A], F16)
            nc.sync.dma_start(
                out=et[:, :, :],
                in_=d_enc_t[:, :].rearrange("(k p) n -> p k n", p=128))
            nc.sync.dma_start(
                out=ewt[:, :, :],
                in_=d_ewt[:, :].rearrange("(k p) a -> p k a", p=128))
            for at in range(4):                      # a-tile = out m-tile
                for ch in range(4):                  # 1568 = 4*392
                    pt = sep.tile([128, 392], F32, tag="sep")
                    for k in range(4):
                        nc.tensor.matmul(
                            pt[:, :],
                            ewt[:, k, at * 128:(at + 1) * 128],
                            et[:, k, ch * 392:(ch + 1) * 392],
                            start=(k == 0), stop=(k == 3))
                    nc.vector.tensor_scalar_add(
                        out=ept[:, at, ch * 392:(ch + 1) * 392],
                        in0=pt[:, :], scalar1=encb[:, at:at + 1])

        # load remaining residents (batched DMAs)
        nc.sync.dma_start(out=wc[:, :, :],
                          in_=d_wc[:, :].rearrange("(k p) g -> p k g", p=128))
        nc.sync.dma_start(out=enr[:, :, :],
                          in_=d_enc_r[:, :, :].rearrange("j p h -> p j h"))
        nc.sync.dma_start(out=dwt[:, :, :],
                          in_=d_dwt[:, :].rearrange("(k p) a -> p k a", p=128))
        for ch in range(NPRE):                    # stream during recurrence
            nc.sync.dma_start(
                out=fcpre[:, ch, :, :],
                in_=d_fcw[:, ch * VC:(ch + 1) * VC].rearrange(
                    "(k p) v -> p k v", p=128))

        # -------- recurrence --------
        with tc.tile_pool(name="psm", bufs=2, space="PSUM") as psm, \
             tc.tile_pool(name="ptp", bufs=2, space="PSUM") as ptp, \
             tc.tile_pool(name="psg", bufs=1, space="PSUM") as psgp, \
             tc.tile_pool(name="stp", bufs=2) as stp, \
             tc.tile_pool(name="xp", bufs=1) as xp, \
             tc.tile_pool(name="gxp", bufs=2) as gxp:
            for t in range(T):
                hprev = (lambda at: h0[:, at, :]) if t == 0 else \
                    (lambda at, _t=t: hallt[:, at, (_t - 1) * 8:(_t - 1) * 8 + 8])

                # ---- dec = 2h @ (.5 dec_W)^T ----
                pd = psm.tile([8, 512], F32, tag="sm")
                for k in range(4):
                    nc.tensor.matmul(pd[:, :], hprev(k), dwt[:, k, :],
                                     start=(k == 0), stop=(k == 3))
                dec = stp.tile([8, 512], F16, tag="dec")
                nc.vector.tensor_copy(out=dec[:, :], in_=pd[:, :])

                # ---- dec^T (+dec_b) ----
                dect = stp.tile([128, 4, 8], F16, tag="dect")
                for at in range(4):
                    ptr = ptp.tile([128, 8], F16, tag="tp")
                    nc.tensor.transpose(ptr[:, :],
                                        dec[:, at * 128:(at + 1) * 128],
                                        id8[:, :])
                    nc.vector.tensor_scalar_add(
                        out=dect[:, at, :], in0=ptr[:, :],
                        scalar1=decb[:, at:at + 1])

                # ---- gates: h-part first (only needs h(t-1)) ----
                gx = gxp.tile([8, 4 * H], F16, tag="gx")
                nc.sync.dma_start(out=gx[:, :],
                                  in_=d_xp[t * 8:(t + 1) * 8, :])
                psg = psgp.tile([8, 4 * H], F32, tag="gates")
                for ch in range(4):
                    sl = slice(ch * 512, (ch + 1) * 512)
                    for k in range(4):
                        nc.tensor.matmul(psg[:, sl], hprev(k),
                                         wc[:, 4 + k, sl],
                                         start=(k == 0), stop=False)
                    nc.tensor.matmul(psg[:, sl], id8[:, :], gx[:, sl],
                                     start=False, stop=False)

                # ---- attention, pipelined over two half-batches ----
                xts = []
                for at in range(4):
                    xts.append(xp.tile([128, BN], F16, tag=f"x{at}",
                                       name=f"xt{t}_{at}"))
                ct = stp.tile([128, 4, 8], F16, tag="ct")
                for h in range(2):
                    hsl = slice(h * 4 * N, (h + 1) * 4 * N)
                    for at in range(4):
                        xt = xts[at]
                        eng = (nc.gpsimd if at == (3 if h == 0 else 1)
                               else nc.vector)
                        eng.tensor_add(
                            out=xt[:, hsl].rearrange(
                                "p (b n) -> p b n", n=N),
                            in0=ept[:, at, hsl].rearrange(
                                "p (b n) -> p b n", n=N),
                            in1=_bcast(dect[:, at, h * 4:(h + 1) * 4], N))
                        nc.scalar.activation(out=xt[:, hsl],
                                             in_=xt[:, hsl], func=TANH)
                    psc = psm.tile([4, N], F32, tag="sm")
                    for bl in range(4):
                        b = 4 * h + bl
                        for at in range(4):
                            nc.tensor.matmul(
                                psc[:, :], ewm[:, at, bl, :],
                                xts[at][:, b * N:(b + 1) * N],
                                start=(bl == 0 and at == 0),
                                stop=(bl == 3 and at == 3))
                    atw = stp.tile([4, N], F16, tag=f"atw{h}")
                    zs = stp.tile([4, 1], F32, tag=f"zs{h}")
                    nc.scalar.activation(out=atw[:, :], in_=psc[:, :],
                                         func=EXP, accum_out=zs[:, 0:1])
                    rz = stp.tile([4, 1], F32, tag=f"rz{h}")
                    nc.vector.reciprocal(out=rz[:, :], in_=zs[:, :])

                    p1 = ptp.tile([128, 4], F16, tag="tp")
                    nc.tensor.transpose(p1[:, :], atw[:, 0:128],
                                        id8[0:4, 0:4])
                    dg = atm[:, h, :, :]
                    nc.vector.tensor_copy(
                        out=bass.AP(tensor=dg.tensor, offset=dg.offset,
                                    ap=[dg.ap[0], [5, 4]]),
                        in_=p1[:, :])
                    p2 = ptp.tile([128, 4], F16, tag="tp")
                    nc.tensor.transpose(p2[0:N - 128, :], atw[:, 128:N],
                                        id8[0:4, 0:4])
                    d2 = at2m[0:N - 128, h, :, :]
                    nc.vector.tensor_copy(
                        out=bass.AP(tensor=d2.tensor, offset=d2.offset,
                                    ap=[d2.ap[0], [5, 4]]),
                        in_=p2[0:N - 128, :])

                    pc = psm.tile([4, 512], F32, tag="sm")
                    for bl in range(4):
                        b = 4 * h + bl
                        nc.tensor.matmul(pc[:, :], atm[:, h, bl, :],
                                         enr[:, 2 * b, :],
                                         start=(bl == 0), stop=False)
                        nc.tensor.matmul(pc[:, :],
                                         at2m[0:N - 128, h, bl, :],
                                         enr[0:N - 128, 2 * b + 1, :],
                                         start=False, stop=(bl == 3))
                    ctxr = stp.tile([4, H], F16, tag=f"ctxr{h}")
                    nc.vector.tensor_scalar_mul(
                        out=ctxr[:, :], in0=pc[:, :], scalar1=rz[:, :])
                    for at in range(4):
                        ptr = ptp.tile([128, 4], F16, tag="tp")
                        nc.tensor.transpose(
                            ptr[:, :], ctxr[:, at * 128:(at + 1) * 128],
                            id8[0:4, 0:4])
                        nc.vector.tensor_copy(
                            out=ct[:, at, h * 4:(h + 1) * 4],
                            in_=ptr[:, :])

                # ---- gates: ctx-part ----
                for ch in range(4):
                    sl = slice(ch * 512, (ch + 1) * 512)
                    for k in range(4):
                        nc.tensor.matmul(psg[:, sl], ct[:, k, :],
                                         wc[:, k, sl],
                                         start=False, stop=(k == 3))

                # ---- pointwise (i,f,o,g; sigmoid via tanh) ----
                th = stp.tile([8, 3 * H], F16, tag="th")
                nc.scalar.activation(out=th[:, :], in_=psg[:, 0:3 * H],
                                     func=TANH, scale=0.5)
                thg = stp.tile([8, H], F16, tag="thg")
                nc.scalar.activation(out=thg[:, :], in_=psg[:, 3 * H:4 * H],
                                     func=TANH)
                a2 = stp.tile([8, H], F32, tag="a2")
                nc.vector.scalar_tensor_tensor(
                    out=a2[:, :], in0=th[:, H:2 * H], scalar=1.0,
                    in1=c2[:, :], op0=ADD, op1=MULT)
                bb = stp.tile([8, H], F32, tag="bb")
                nc.vector.scalar_tensor_tensor(
                    out=bb[:, :], in0=th[:, 0:H], scalar=1.0,
                    in1=thg[:, :], op0=ADD, op1=MULT)
                nc.vector.scalar_tensor_tensor(
                    out=c2[:, :], in0=a2[:, :], scalar=0.5,
                    in1=bb[:, :], op0=MULT, op1=ADD)
                thc = stp.tile([8, H], F32, tag="thc")
                nc.scalar.activation(out=thc[:, :], in_=c2[:, :],
                                     func=TANH, scale=0.5)
                h2r = stp.tile([8, H], F16, tag="h2r")
                nc.vector.scalar_tensor_tensor(
                    out=h2r[:, :], in0=th[:, 2 * H:3 * H], scalar=1.0,
                    in1=thc[:, :], op0=ADD, op1=MULT)

                # ---- h^T into hallt[:, :, t*8:(t+1)*8] ----
                for at in range(4):
                    ptr = ptp.tile([128, 8], F16, tag="tp")
                    nc.tensor.transpose(ptr[:, :],
                                        h2r[:, at * 128:(at + 1) * 128],
                                        id8[:, :])
                    nc.vector.tensor_copy(
                        out=hallt[:, at, t * 8:(t + 1) * 8], in_=ptr[:, :])

        # -------- fc --------
        with tc.tile_pool(name="fw", bufs=8) as fwp, \
             tc.tile_pool(name="fb", bufs=2) as fbp, \
             tc.tile_pool(name="fo", bufs=3) as fop, \
             tc.tile_pool(name="pf", bufs=3, space="PSUM") as pfp:
            fbt = None
            for ch in range(NCH):
                vsl = slice(ch * VC, (ch + 1) * VC)
                if ch % 8 == 0:
                    fbt = fbp.tile([1, 8, VC], F16, tag="fb",
                                   name=f"fbt{ch}")
                    nc.sync.dma_start(
                        out=fbt[:, :, :],
                        in_=d_fcb[0:1, ch * VC:(ch + 8) * VC].rearrange(
                            "p (c v) -> p c v", v=VC))
                fbc = fbt[0:1, ch % 8, :]
                if ch < NPRE:
                    fws = fcpre[:, ch, :, :]
                else:
                    fws = fwp.tile([128, 4, VC], F16, tag="fw",
                                   name=f"fw{ch}")
                    nc.sync.dma_start(
                        out=fws[:, :, :],
                        in_=d_fcw[:, vsl].rearrange(
                            "(k p) v -> p k v", p=128))
                for m in range(2):
                    mr = 128 if m == 0 else BT - 128
                    pf = pfp.tile([128, VC], F32, tag="pf")
                    for k in range(4):
                        nc.tensor.matmul(
                            pf[0:mr, :],
                            hallt[:, k, m * 128:m * 128 + mr],
                            fws[:, k, :], start=(k == 0), stop=False)
                    nc.tensor.matmul(pf[0:mr, :], ones[0:1, 0:mr],
                                     fbc, start=False, stop=True)
                    fo = fop.tile([128, VC], F32, tag="fo")
                    nc.vector.tensor_copy(out=fo[0:mr, :], in_=pf[0:mr, :])
                    nc.sync.dma_start(
                        out=d_out[m * 128:m * 128 + mr, vsl],
                        in_=fo[0:mr, :])


_PROGRAM = None


def kernel(**inputs) -> np.ndarray:
    global _PROGRAM
    if _PROGRAM is None:
        _PROGRAM = build_program()
    in_maps = [prep_core(c, inputs) for c in range(NC)]
    res = run_bass_kernel_spmd(_PROGRAM, in_maps, core_ids=list(range(NC)))
    out = np.zeros((B, L, V), np.float32)
    for c in range(NC):
        lg = res.results[c]["logits"].reshape(T, BS, V)
        out[c * BS:(c + 1) * BS, 1:, :] = lg.transpose(1, 0, 2)
    return out


if __name__ == "__main__":
    import reference
    ins = {k: np.asarray(v) for k, v in reference.setup_inputs().items()}
    got = kernel(**ins)
    exp = np.asarray(reference.reference(**reference.setup_inputs()))
    err = np.abs(got - exp).max() / (np.abs(exp).max() + 1e-12)
    print("Relative error:", err)
